# revision 1
# baseline (speedup 1.0000x reference)
"""DeepseekMoE Trainium2 kernel (expert-parallel over 8 NeuronCores).

Strategy:
  - Host: gate (softmax + top-6), capacity dispatch bookkeeping, packing.
  - Device (SPMD, 8 cores): core c owns experts [8c, 8c+8). Grouped SwiGLU
    expert GEMMs in token-stationary orientation (weights stream through the
    PE array as the moving operand, so cost is independent of per-expert
    token count). Shared-expert SwiGLU is tensor-parallel over its
    intermediate dim (352 columns per core). All matmuls run in float32r
    (full-rate fp32 path, ~1e-4 matmul rel-err).
  - Host: weighted combine of expert rows + sum of shared partials.
"""
import os
import sys
import numpy as np

for _p in ("/opt/trn_rl_repo", "/root/.axon_site/_ro/trn_rl_repo"):
    if os.path.isdir(_p) and _p not in sys.path:
        sys.path.insert(0, _p)

E, K, H, I = 64, 6, 2048, 1408
NS = 2
IS = NS * I          # 2816
CAP = 256
T = 1024             # tokens
NCORES = 8
EPC = E // NCORES    # experts per core = 8
ISC = IS // NCORES   # shared intermediate per core = 352

_PROG_CACHE = {}


def _build_program(c_pad, row_tiles, reps=1):
    """Build the SPMD Bass program. Structure depends only on (c_pad, row_tiles)."""
    import concourse.bacc as bacc
    import concourse.mybir as mybir
    from concourse.tile import TileContext
    from concourse import masks

    f32 = mybir.dt.float32
    f32r = mybir.dt.float32r
    Silu = mybir.ActivationFunctionType.Silu

    IC = [(0, 512), (512, 512), (1024, 384)]          # I chunks (gate/up psum)
    KD = [(0, 128), (128, 128), (256, 96)]            # shared down K tiles over 352

    nc = bacc.Bacc()
    xt_d = nc.declare_dram_parameter("xt", [EPC, 128, 16 * c_pad], f32, isOutput=False)
    wg_d = nc.declare_dram_parameter("wg", [EPC, H, I], f32, isOutput=False)
    wu_d = nc.declare_dram_parameter("wu", [EPC, H, I], f32, isOutput=False)
    wd_d = nc.declare_dram_parameter("wd", [EPC, I, H], f32, isOutput=False)
    scl_d = nc.declare_dram_parameter("scl", [EPC, c_pad], f32, isOutput=False)
    xsh_d = nc.declare_dram_parameter("xsh", [16, 128, T], f32, isOutput=False)
    swg_d = nc.declare_dram_parameter("swg", [H, ISC], f32, isOutput=False)
    swu_d = nc.declare_dram_parameter("swu", [H, ISC], f32, isOutput=False)
    swd_d = nc.declare_dram_parameter("swd", [ISC, H], f32, isOutput=False)
    y_d = nc.declare_dram_parameter("y", [EPC, c_pad, H], f32, isOutput=True)
    ysh_d = nc.declare_dram_parameter("ysh", [T, H], f32, isOutput=True)

    wg_r = wg_d.bitcast(f32r)
    wu_r = wu_d.bitcast(f32r)
    wd_r = wd_d.bitcast(f32r)
    xt_r = xt_d.bitcast(f32r)
    swg_r = swg_d.bitcast(f32r)
    swu_r = swu_d.bitcast(f32r)
    swd_r = swd_d.bitcast(f32r)
    xsh_r = xsh_d.bitcast(f32r)

    with TileContext(nc) as tc:
        with (
            tc.tile_pool(name="const", bufs=1) as constp,
            tc.tile_pool(name="xt", bufs=2) as xtp,
            tc.tile_pool(name="w", bufs=4) as wp,
            tc.tile_pool(name="wd", bufs=2) as wdp,
            tc.tile_pool(name="hid", bufs=3) as hidp,
            tc.tile_pool(name="hidT", bufs=2) as hidTp,
            tc.tile_pool(name="ysb", bufs=3) as ysbp,
            tc.tile_pool(name="scl", bufs=2) as sclp,
            tc.tile_pool(name="swres", bufs=1) as swresp,
            tc.tile_pool(name="psg", bufs=1, space="PSUM") as psg,
            tc.tile_pool(name="psu", bufs=1, space="PSUM") as psu,
            tc.tile_pool(name="pstr", bufs=2, space="PSUM") as pstr,
            tc.tile_pool(name="psy", bufs=1, space="PSUM") as psy,
        ):
            ident = constp.tile([128, 128], f32)
            masks.make_identity(nc, ident[:])

            for _rep in range(reps):
                _expert_shared_body(nc, tc, mybir, c_pad, row_tiles,
                                    xt_r, wg_r, wu_r, wd_r, scl_d, xsh_r,
                                    swg_r, swu_r, swd_r, y_d, ysh_d, ident,
                                    xtp, wp, wdp, hidp, hidTp, ysbp, sclp, swresp,
                                    psg, psu, pstr, psy)
    nc.compile()
    return nc


def _expert_shared_body(nc, tc, mybir, c_pad, row_tiles,
                        xt_r, wg_r, wu_r, wd_r, scl_d, xsh_r,
                        swg_r, swu_r, swd_r, y_d, ysh_d, ident,
                        xtp, wp, wdp, hidp, hidTp, ysbp, sclp, swresp,
                        psg, psu, pstr, psy):
    f32 = mybir.dt.float32
    f32r = mybir.dt.float32r
    Silu = mybir.ActivationFunctionType.Silu
    IC = [(0, 512), (512, 512), (1024, 384)]
    KD = [(0, 128), (128, 128), (256, 96)]
    if True:
        if True:
            # ---------------- expert phase ----------------
            for j in range(EPC):
                xt_sb = xtp.tile([128, 16 * c_pad], f32r, tag="xt")
                nc.sync.dma_start(out=xt_sb[:], in_=xt_r[j])
                scl_sb = sclp.tile([128, 1], f32, tag="scl")
                nc.sync.dma_start(out=scl_sb[:c_pad, 0:1], in_=scl_d[j].rearrange("(c o) -> c o", o=1))

                for (r0, rm) in row_tiles:
                    # gate/up one I-chunk at a time: 1 PSUM bank for g, 1 for u
                    hidT_sb = hidTp.tile([128, 11 * 128], f32r, tag="hidT")
                    for (i0, ci) in IC:
                        g_ps = psg.tile([128, 512], f32, tag="g")
                        u_ps = psu.tile([128, 512], f32, tag="u")
                        for h in range(16):
                            wg_sb = wp.tile([128, 512], f32r, tag="wg")
                            wu_sb = wp.tile([128, 512], f32r, tag="wu")
                            nc.sync.dma_start(out=wg_sb[:, :ci], in_=wg_r[j, h * 128:(h + 1) * 128, i0:i0 + ci])
                            nc.sync.dma_start(out=wu_sb[:, :ci], in_=wu_r[j, h * 128:(h + 1) * 128, i0:i0 + ci])
                            lhsT = xt_sb[:, h * c_pad + r0: h * c_pad + r0 + rm]
                            nc.tensor.matmul(g_ps[:rm, :ci], lhsT, wg_sb[:, :ci],
                                             start=(h == 0), stop=(h == 15))
                            nc.tensor.matmul(u_ps[:rm, :ci], lhsT, wu_sb[:, :ci],
                                             start=(h == 0), stop=(h == 15))
                        hid_sb = hidp.tile([128, 512], f32, tag="hid")
                        nc.scalar.activation(hid_sb[:rm, :ci], g_ps[:rm, :ci], Silu)
                        nc.vector.tensor_mul(hid_sb[:rm, :ci], hid_sb[:rm, :ci], u_ps[:rm, :ci])
                        for t in range(ci // 128):
                            tr_ps = pstr.tile([128, 128], f32, tag="tr")
                            nc.tensor.transpose(tr_ps[:, :rm], hid_sb[:rm, t * 128:(t + 1) * 128],
                                                ident[:rm, :rm])
                            k2 = (i0 // 128) + t
                            nc.vector.tensor_copy(hidT_sb[:, k2 * 128: k2 * 128 + rm], tr_ps[:, :rm])
                    # down projection
                    y_ps = psy.tile([128, H], f32, tag="y")
                    for k2 in range(11):
                        wd_sb = wdp.tile([128, H], f32r, tag="wd")
                        nc.sync.dma_start(out=wd_sb[:], in_=wd_r[j, k2 * 128:(k2 + 1) * 128, :])
                        for n in range(4):
                            nc.tensor.matmul(y_ps[:rm, n * 512:(n + 1) * 512],
                                             hidT_sb[:, k2 * 128: k2 * 128 + rm],
                                             wd_sb[:, n * 512:(n + 1) * 512],
                                             start=(k2 == 0), stop=(k2 == 10))
                    y_sb = ysbp.tile([128, H], f32, tag="ysb")
                    for n in range(4):
                        nc.vector.tensor_scalar_mul(y_sb[:rm, n * 512:(n + 1) * 512],
                                                    y_ps[:rm, n * 512:(n + 1) * 512],
                                                    scl_sb[r0:r0 + rm, 0:1])
                    nc.sync.dma_start(out=y_d[j, r0:r0 + rm, :], in_=y_sb[:rm, :])

            # ---------------- shared expert phase ----------------
            # resident sliced weights
            swg_sb = swresp.tile([128, 16 * ISC], f32r, tag="swg")
            swu_sb = swresp.tile([128, 16 * ISC], f32r, tag="swu")
            nc.sync.dma_start(out=swg_sb[:].rearrange("p (h i) -> p h i", h=16),
                              in_=swg_r.rearrange("(h p) i -> p h i", p=128))
            nc.sync.dma_start(out=swu_sb[:].rearrange("p (h i) -> p h i", h=16),
                              in_=swu_r.rearrange("(h p) i -> p h i", p=128))
            swd_sbs = []
            for t, (k0, km) in enumerate(KD):
                sd = swresp.tile([128, H], f32r, tag=f"swd{t}", name=f"swd{t}")
                nc.sync.dma_start(out=sd[:km, :], in_=swd_r[k0:k0 + km, :])
                swd_sbs.append(sd)

            for th in range(2):  # token halves of 512
                # gT/uT: [ISC, 512] computed one 128-row i-tile at a time
                hsh_tiles = []
                for it, (k0, km) in enumerate(KD):
                    g_ps = psg.tile([128, 512], f32, tag="g")
                    u_ps = psu.tile([128, 512], f32, tag="u")
                    for h in range(16):
                        xts = wp.tile([128, 512], f32r, tag="xts")
                        nc.sync.dma_start(out=xts[:], in_=xsh_r[h, :, th * 512:(th + 1) * 512])
                        lhs_g = swg_sb[:, h * ISC + k0: h * ISC + k0 + km]
                        lhs_u = swu_sb[:, h * ISC + k0: h * ISC + k0 + km]
                        nc.tensor.matmul(g_ps[:km, :], lhs_g, xts[:],
                                         start=(h == 0), stop=(h == 15))
                        nc.tensor.matmul(u_ps[:km, :], lhs_u, xts[:],
                                         start=(h == 0), stop=(h == 15))
                    hsh = hidp.tile([128, 512], f32, tag="hid")
                    nc.scalar.activation(hsh[:km, :], g_ps[:km, :], Silu)
                    nc.vector.tensor_mul(hsh[:km, :], hsh[:km, :], u_ps[:km, :])
                    hshr = hidp.tile([128, 512], f32r, tag="hshr")
                    nc.vector.tensor_copy(hshr[:km, :], hsh[:km, :])
                    hsh_tiles.append(hshr)
                for mt in range(4):  # 128-token sub-tiles
                    ysh_ps = psy.tile([128, H], f32, tag="y")
                    for it, (k0, km) in enumerate(KD):
                        for n in range(4):
                            nc.tensor.matmul(ysh_ps[:, n * 512:(n + 1) * 512],
                                             hsh_tiles[it][:km, mt * 128:(mt + 1) * 128],
                                             swd_sbs[it][:km, n * 512:(n + 1) * 512],
                                             start=(it == 0), stop=(it == 2))
                    ysh_sb = ysbp.tile([128, H], f32, tag="ysb")
                    nc.vector.tensor_copy(ysh_sb[:], ysh_ps[:])
                    nc.sync.dma_start(out=ysh_d[th * 512 + mt * 128: th * 512 + (mt + 1) * 128, :],
                                      in_=ysh_sb[:])


class _Exec:
    """Compile once; run the SPMD program on 8 cores with device-resident inputs."""

    def __init__(self, nc):
        import jax
        import jax.numpy as jnp
        import concourse.mybir as mybir
        from concourse import bass2jax
        from concourse.bass2jax import shard_map, Mesh, PartitionSpec

        bass2jax.install_neuronx_cc_hook()
        self._jax = jax
        self._jnp = jnp
        self.nc = nc
        partition_name = nc.partition_id_tensor.name if nc.partition_id_tensor else None

        in_names, out_names, out_avals = [], [], []
        for alloc in nc.m.functions[0].allocations:
            if not isinstance(alloc, mybir.MemoryLocationSet):
                continue
            name = alloc.memorylocations[0].name
            if alloc.kind == "ExternalInput":
                if name != partition_name:
                    in_names.append(name)
            elif alloc.kind == "ExternalOutput":
                shape = tuple(alloc.tensor_shape)
                dtype = mybir.dt.np(alloc.dtype)
                out_names.append(name)
                out_avals.append(jax.core.ShapedArray(shape, dtype))
        self.param_names = list(in_names)
        self.out_names = list(out_names)
        self.out_avals = out_avals
        n_params = len(in_names)
        n_outs = len(out_names)
        bir_in_names = in_names + out_names + ([partition_name] if partition_name else [])

        devices = jax.devices()[:NCORES]
        mesh = Mesh(np.asarray(devices), ("core",))
        self.mesh = mesh
        self.pspec = PartitionSpec("core")

        def _body_k(k):
            def _body(*args):
                params = list(args[:n_params])
                outs = list(args[n_params:])
                for _ in range(k):
                    operands = params + outs
                    if partition_name is not None:
                        operands.append(bass2jax.partition_id_tensor())
                    outs = list(bass2jax._bass_exec_p.bind(
                        *operands,
                        out_avals=tuple(out_avals),
                        in_names=tuple(bir_in_names),
                        out_names=tuple(out_names),
                        lowering_input_output_aliases=(),
                        sim_require_finite=True,
                        sim_require_nnan=True,
                        nc=nc,
                    ))
                return tuple(outs)
            in_specs = (PartitionSpec("core"),) * (n_params + n_outs)
            out_specs = (PartitionSpec("core"),) * n_outs
            return jax.jit(
                shard_map(_body, mesh=mesh, in_specs=in_specs, out_specs=out_specs,
                          check_rep=False),
                keep_unused=True,
            )

        self._body_k = _body_k
        self._fns = {}
        self.dev_in = None

    def _fn(self, k):
        if k not in self._fns:
            self._fns[k] = self._body_k(k)
        return self._fns[k]

    def put_inputs(self, in_maps):
        import jax
        from jax.sharding import NamedSharding
        sh = NamedSharding(self.mesh, self.pspec)
        self.dev_in = [
            jax.device_put(
                np.concatenate([np.asarray(m[name]) for m in in_maps], axis=0), sh)
            for name in self.param_names
        ]
        self._zeros = [
            jax.device_put(np.zeros((NCORES * av.shape[0], *av.shape[1:]), av.dtype), sh)
            for av in self.out_avals
        ]

    def run(self, k=1):
        outs = self._fn(k)(*self.dev_in, *self._zeros)
        for o in outs:
            o.block_until_ready()
        return outs

    def results(self, outs):
        res = []
        for c in range(NCORES):
            d = {}
            for i, name in enumerate(self.out_names):
                av = self.out_avals[i]
                d[name] = np.asarray(outs[i]).reshape(NCORES, *av.shape)[c]
            res.append(d)
        return res

    def bench(self, iters=10, warmup=2):
        import time as _t
        for _ in range(warmup):
            self.run(1)
        walls = []
        for _ in range(iters):
            t0 = _t.time()
            self.run(1)
            walls.append((_t.time() - t0) * 1e9)
        walls.sort()
        return walls[0], walls[len(walls) // 2], walls


LAST_EXEC = None
LAST_EXEC_NS = None


def null_overhead_ns(iters=10):
    """Dispatch+sync overhead of a launch, measured with a trivial kernel."""
    import concourse.bacc as bacc
    import concourse.mybir as mybir
    from concourse.tile import TileContext

    key = "null"
    if key not in _PROG_CACHE:
        nc = bacc.Bacc()
        xin = nc.declare_dram_parameter("x", [128, 128], mybir.dt.float32, isOutput=False)
        out = nc.declare_dram_parameter("o", [128, 128], mybir.dt.float32, isOutput=True)
        with TileContext(nc) as tc:
            with tc.tile_pool(name="sb", bufs=1) as sb:
                t = sb.tile([128, 128], mybir.dt.float32)
                nc.sync.dma_start(out=t[:], in_=xin[:])
                nc.sync.dma_start(out=out[:], in_=t[:])
        nc.compile()
        ex = _Exec(nc)
        ex.put_inputs([{"x": np.zeros((128, 128), np.float32)} for _ in range(NCORES)])
        _PROG_CACHE[key] = ex
    ex = _PROG_CACHE[key]
    mn, med, _ = ex.bench(iters=iters)
    return mn


def _get_program(c_pad, row_tiles):
    key = (c_pad, tuple(row_tiles))
    if key not in _PROG_CACHE:
        nc = _build_program(c_pad, row_tiles)
        _PROG_CACHE[key] = _Exec(nc)
    return _PROG_CACHE[key]


def kernel(hidden_states, gate_w, w_gate, w_up, w_down, sw_gate, sw_up, sw_down):
    global LAST_EXEC
    orig_shape = hidden_states.shape
    x = np.ascontiguousarray(np.asarray(hidden_states, np.float32).reshape(-1, H))
    t_tokens = x.shape[0]
    assert t_tokens == T, f"kernel compiled for T={T}, got {t_tokens}"

    # ---- host gate: softmax + top-6 (matches jax.lax.top_k tie order) ----
    logits = x @ np.asarray(gate_w, np.float32).T
    m = logits.max(axis=-1, keepdims=True)
    ex = np.exp(logits - m)
    scores = ex / ex.sum(axis=-1, keepdims=True)
    order = np.argsort(-scores, axis=-1, kind="stable")[:, :K]
    topk_w = np.take_along_axis(scores, order, axis=1)
    flat_i = order.reshape(-1).astype(np.int64)
    flat_w = topk_w.reshape(-1).astype(np.float32)
    token_ids = np.arange(T * K, dtype=np.int64) // K

    # ---- rank within expert (stable by slot index) ----
    sort_order = np.argsort(flat_i, kind="stable")
    sorted_e = flat_i[sort_order]
    counts = np.bincount(flat_i, minlength=E)
    starts = np.concatenate([[0], np.cumsum(counts)[:-1]])
    pos_sorted = np.arange(T * K) - starts[sorted_e]
    pos = np.empty(T * K, np.int64)
    pos[sort_order] = pos_sorted

    max_load = int(min(counts.max(), CAP))
    c_pad = max(128, ((max_load + 127) // 128) * 128)
    c_pad = min(c_pad, CAP)
    row_tiles = []
    r0 = 0
    while r0 < c_pad:
        row_tiles.append((r0, min(128, c_pad - r0)))
        r0 += 128

    # ---- pack per-core inputs ----
    xT = np.ascontiguousarray(x.T)          # [H, T]
    xsh = xT.reshape(16, 128, T)            # [h, p, tokens] — replicated to all cores

    w_gate = np.asarray(w_gate, np.float32)
    w_up = np.asarray(w_up, np.float32)
    w_down = np.asarray(w_down, np.float32)
    sw_gate = np.asarray(sw_gate, np.float32)
    sw_up = np.asarray(sw_up, np.float32)
    sw_down = np.asarray(sw_down, np.float32)

    tok_of = np.zeros((E, c_pad), np.int64)
    nrow = np.zeros(E, np.int64)
    for e in range(E):
        cnt = int(min(counts[e], c_pad))
        seg = sort_order[starts[e]: starts[e] + cnt]
        tok_of[e, :cnt] = token_ids[seg]
        nrow[e] = cnt
    slot_w = np.zeros((E, c_pad), np.float32)
    for e in range(E):
        cnt = int(nrow[e])
        seg = sort_order[starts[e]: starts[e] + cnt]
        slot_w[e, :cnt] = flat_w[seg]

    in_maps = []
    for c in range(NCORES):
        eids = np.arange(c * EPC, (c + 1) * EPC)
        xt_c = np.zeros((EPC, 128, 16 * c_pad), np.float32)
        for jj, e in enumerate(eids):
            cnt = int(nrow[e])
            rows = x[tok_of[e, :cnt]]                      # [cnt, H]
            buf = np.zeros((c_pad, H), np.float32)
            buf[:cnt] = rows
            # -> [H, c_pad] -> [16, 128, c_pad] -> [128, 16, c_pad]
            xt_c[jj] = buf.T.reshape(16, 128, c_pad).transpose(1, 0, 2).reshape(128, 16 * c_pad)
        in_maps.append({
            "xt": np.ascontiguousarray(xt_c),
            "wg": np.ascontiguousarray(w_gate[eids]),
            "wu": np.ascontiguousarray(w_up[eids]),
            "wd": np.ascontiguousarray(w_down[eids]),
            "scl": np.ascontiguousarray(slot_w[eids]),
            "xsh": xsh,
            "swg": np.ascontiguousarray(sw_gate[:, c * ISC:(c + 1) * ISC]),
            "swu": np.ascontiguousarray(sw_up[:, c * ISC:(c + 1) * ISC]),
            "swd": np.ascontiguousarray(sw_down[c * ISC:(c + 1) * ISC, :]),
        })

    ex = _get_program(c_pad, row_tiles)
    ex.put_inputs(in_maps)
    outs = ex.run(1)
    results = ex.results(outs)
    LAST_EXEC = ex

    # ---- host combine ----
    y_all = np.concatenate([results[c]["y"] for c in range(NCORES)], axis=0)  # [E, c_pad, H]
    shared = np.zeros((T, H), np.float64)
    for c in range(NCORES):
        shared += results[c]["ysh"].astype(np.float64)

    mask = (pos < CAP) & (pos < c_pad)
    idx = np.where(mask, pos, 0)
    rows = y_all[flat_i, idx] * mask[:, None].astype(np.float32)   # [T*K, H], pre-scaled
    moe_out = rows.reshape(T, K, H).sum(axis=1)

    out = (moe_out.astype(np.float64) + shared).astype(np.float32)
    return out.reshape(orig_shape)



# revision 15
# speedup vs baseline: 1.0166x; 1.0166x over previous
"""DeepseekMoE Trainium2 kernel (expert-parallel over 8 NeuronCores).

Strategy:
  - Host: gate (softmax + top-6), capacity dispatch bookkeeping, packing.
  - Device (SPMD, 8 cores): core c owns experts [8c, 8c+8). Grouped SwiGLU
    expert GEMMs in token-stationary orientation. All matmul operands are
    bf16 (f32 PSUM accumulation), halving HBM/transfer bytes vs f32.
    Gate+up weights for each (I-chunk, h-tile) are packed side by side on
    the host so each weight DMA is one contiguous [128, ci*2] transfer.
    Shared-expert SwiGLU is tensor-parallel over its intermediate dim
    (352 columns per core).
  - Host: weighted combine of expert rows + sum of shared partials.
"""
import os
import sys
import numpy as np

for _p in ("/opt/trn_rl_repo", "/root/.axon_site/_ro/trn_rl_repo"):
    if os.path.isdir(_p) and _p not in sys.path:
        sys.path.insert(0, _p)

E, K, H, I = 64, 6, 2048, 1408
NS = 2
IS = NS * I          # 2816
CAP = 256
T = 1024             # tokens
NCORES = 8
EPC = E // NCORES    # experts per core = 8
ISC = IS // NCORES   # shared intermediate per core = 352

IC = [(0, 512), (512, 512), (1024, 384)]          # I chunks (gate/up psum)
SIC = [(0, 512), (512, 512), (1024, 512), (1536, 512), (2048, 512), (2560, 256)]
TPC = T // NCORES    # shared-expert tokens per core = 128

_PROG_CACHE = {}


def _build_program(c_pad, row_tiles, reps=1):
    """Build the SPMD Bass program. Structure depends only on (c_pad, row_tiles)."""
    import concourse.bacc as bacc
    import concourse.mybir as mybir
    from concourse.tile import TileContext
    from concourse import masks

    f32 = mybir.dt.float32
    bf16 = mybir.dt.bfloat16

    nc = bacc.Bacc()
    xt_d = nc.declare_dram_parameter("xt", [EPC, 128, 16 * c_pad], bf16, isOutput=False)
    # gate|up weights packed per (I-chunk, h-tile): one contiguous DMA each
    wgu_ds = []
    for ic, (i0, ci) in enumerate(IC):
        wgu_ds.append(nc.declare_dram_parameter(
            f"wgu{ic}", [EPC, 16, 128, 2 * ci], bf16, isOutput=False))
    wd_d = nc.declare_dram_parameter("wd", [EPC, I, H], bf16, isOutput=False)
    scl_d = nc.declare_dram_parameter("scl", [EPC, c_pad], f32, isOutput=False)
    # shared expert: token-sharded — this core's TPC tokens vs FULL shared weights
    xshc_d = nc.declare_dram_parameter("xshc", [128, 16 * TPC], bf16, isOutput=False)
    sgu_ds = []
    for ic, (i0, ci) in enumerate(SIC):
        sgu_ds.append(nc.declare_dram_parameter(
            f"sgu{ic}", [16, 128, 2 * ci], bf16, isOutput=False))
    swd_d = nc.declare_dram_parameter("swd", [IS, H], bf16, isOutput=False)
    y_d = nc.declare_dram_parameter("y", [EPC, c_pad, H], bf16, isOutput=True)
    ysh_d = nc.declare_dram_parameter("ysh", [TPC, H], bf16, isOutput=True)

    with TileContext(nc) as tc:
        with (
            tc.tile_pool(name="const", bufs=1) as constp,
            tc.tile_pool(name="xt", bufs=2) as xtp,
            tc.tile_pool(name="w", bufs=4) as wp,
            tc.tile_pool(name="wd", bufs=2) as wdp,
            tc.tile_pool(name="hid", bufs=3) as hidp,
            tc.tile_pool(name="hidT", bufs=2) as hidTp,
            tc.tile_pool(name="ysb", bufs=3) as ysbp,
            tc.tile_pool(name="scl", bufs=2) as sclp,
            tc.tile_pool(name="psg", bufs=1, space="PSUM") as psg,
            tc.tile_pool(name="psu", bufs=1, space="PSUM") as psu,
            tc.tile_pool(name="pstr", bufs=2, space="PSUM") as pstr,
            tc.tile_pool(name="psy", bufs=1, space="PSUM") as psy,
        ):
            ident = constp.tile([128, 128], f32)
            masks.make_identity(nc, ident[:])

            for _rep in range(reps):
                _expert_shared_body(nc, tc, mybir, c_pad, row_tiles,
                                    xt_d, wgu_ds, wd_d, scl_d, xshc_d,
                                    sgu_ds, swd_d, y_d, ysh_d, ident,
                                    xtp, wp, wdp, hidp, hidTp, ysbp, sclp,
                                    psg, psu, pstr, psy)
    nc.compile()
    return nc


def _expert_shared_body(nc, tc, mybir, c_pad, row_tiles,
                        xt_d, wgu_ds, wd_d, scl_d, xshc_d,
                        sgu_ds, swd_d, y_d, ysh_d, ident,
                        xtp, wp, wdp, hidp, hidTp, ysbp, sclp,
                        psg, psu, pstr, psy):
    f32 = mybir.dt.float32
    bf16 = mybir.dt.bfloat16
    Silu = mybir.ActivationFunctionType.Silu

    # ---------------- expert phase ----------------
    for j in range(EPC):
        xt_sb = xtp.tile([128, 16 * c_pad], bf16, tag="xt")
        nc.sync.dma_start(out=xt_sb[:], in_=xt_d[j])
        scl_sb = sclp.tile([128, 1], f32, tag="scl")
        nc.sync.dma_start(out=scl_sb[:c_pad, 0:1], in_=scl_d[j].rearrange("(c o) -> c o", o=1))

        for (r0, rm) in row_tiles:
            hidT_sb = hidTp.tile([128, 11 * 128], bf16, tag="hidT")
            for ic, (i0, ci) in enumerate(IC):
                g_ps = psg.tile([128, 512], f32, tag="g")
                u_ps = psu.tile([128, 512], f32, tag="u")
                for h in range(16):
                    wgu_sb = wp.tile([128, 1024], bf16, tag="wgu")
                    nc.sync.dma_start(out=wgu_sb[:, :2 * ci], in_=wgu_ds[ic][j, h])
                    lhsT = xt_sb[:, h * c_pad + r0: h * c_pad + r0 + rm]
                    nc.tensor.matmul(g_ps[:rm, :ci], lhsT, wgu_sb[:, :ci],
                                     start=(h == 0), stop=(h == 15))
                    nc.tensor.matmul(u_ps[:rm, :ci], lhsT, wgu_sb[:, ci:2 * ci],
                                     start=(h == 0), stop=(h == 15))
                hid_sb = hidp.tile([128, 512], f32, tag="hid")
                nc.scalar.activation(hid_sb[:rm, :ci], g_ps[:rm, :ci], Silu)
                nc.vector.tensor_mul(hid_sb[:rm, :ci], hid_sb[:rm, :ci], u_ps[:rm, :ci])
                for t in range(ci // 128):
                    tr_ps = pstr.tile([128, 128], f32, tag="tr")
                    nc.tensor.transpose(tr_ps[:, :rm], hid_sb[:rm, t * 128:(t + 1) * 128],
                                        ident[:rm, :rm])
                    k2 = (i0 // 128) + t
                    nc.vector.tensor_copy(hidT_sb[:, k2 * 128: k2 * 128 + rm], tr_ps[:, :rm])
            # down projection
            y_ps = psy.tile([128, H], f32, tag="y")
            for k2 in range(11):
                wd_sb = wdp.tile([128, H], bf16, tag="wd")
                nc.sync.dma_start(out=wd_sb[:], in_=wd_d[j, k2 * 128:(k2 + 1) * 128, :])
                for n in range(4):
                    nc.tensor.matmul(y_ps[:rm, n * 512:(n + 1) * 512],
                                     hidT_sb[:, k2 * 128: k2 * 128 + rm],
                                     wd_sb[:, n * 512:(n + 1) * 512],
                                     start=(k2 == 0), stop=(k2 == 10))
            y_sb = ysbp.tile([128, H], bf16, tag="ysb")
            for n in range(4):
                nc.vector.tensor_scalar_mul(y_sb[:rm, n * 512:(n + 1) * 512],
                                            y_ps[:rm, n * 512:(n + 1) * 512],
                                            scl_sb[r0:r0 + rm, 0:1])
            nc.sync.dma_start(out=y_d[j, r0:r0 + rm, :], in_=y_sb[:rm, :])

    # ---------------- shared expert phase (token-sharded) ----------------
    # this core's TPC tokens through the FULL shared SwiGLU; same loop shape
    # as one expert with I -> IS (22 k-tiles), no per-row scaling.
    NK2 = IS // 128  # 22
    xsh_sb = xtp.tile([128, 16 * TPC], bf16, tag="xt")
    nc.sync.dma_start(out=xsh_sb[:], in_=xshc_d[:])
    hidT2 = hidTp.tile([128, NK2 * 128], bf16, tag="hidT2")
    for ic, (i0, ci) in enumerate(SIC):
        g_ps = psg.tile([128, 512], f32, tag="g")
        u_ps = psu.tile([128, 512], f32, tag="u")
        for h in range(16):
            sgu_sb = wp.tile([128, 1024], bf16, tag="wgu")
            nc.sync.dma_start(out=sgu_sb[:, :2 * ci], in_=sgu_ds[ic][h])
            lhsT = xsh_sb[:, h * TPC: h * TPC + TPC]
            nc.tensor.matmul(g_ps[:TPC, :ci], lhsT, sgu_sb[:, :ci],
                             start=(h == 0), stop=(h == 15))
            nc.tensor.matmul(u_ps[:TPC, :ci], lhsT, sgu_sb[:, ci:2 * ci],
                             start=(h == 0), stop=(h == 15))
        hid_sb = hidp.tile([128, 512], f32, tag="hid")
        nc.scalar.activation(hid_sb[:TPC, :ci], g_ps[:TPC, :ci], Silu)
        nc.vector.tensor_mul(hid_sb[:TPC, :ci], hid_sb[:TPC, :ci], u_ps[:TPC, :ci])
        for t in range(ci // 128):
            tr_ps = pstr.tile([128, 128], f32, tag="tr")
            nc.tensor.transpose(tr_ps[:, :TPC], hid_sb[:TPC, t * 128:(t + 1) * 128],
                                ident[:TPC, :TPC])
            k2 = (i0 // 128) + t
            nc.vector.tensor_copy(hidT2[:, k2 * 128: k2 * 128 + TPC], tr_ps[:, :TPC])
    ysh_ps = psy.tile([128, H], f32, tag="y")
    for k2 in range(NK2):
        swd_sb = wdp.tile([128, H], bf16, tag="wd")
        nc.sync.dma_start(out=swd_sb[:], in_=swd_d[k2 * 128:(k2 + 1) * 128, :])
        for n in range(4):
            nc.tensor.matmul(ysh_ps[:TPC, n * 512:(n + 1) * 512],
                             hidT2[:, k2 * 128: k2 * 128 + TPC],
                             swd_sb[:, n * 512:(n + 1) * 512],
                             start=(k2 == 0), stop=(k2 == NK2 - 1))
    ysh_sb = ysbp.tile([128, H], bf16, tag="ysb")
    nc.vector.tensor_copy(ysh_sb[:TPC, :], ysh_ps[:TPC, :])
    nc.sync.dma_start(out=ysh_d[:, :], in_=ysh_sb[:TPC, :])


class _Exec:
    """Compile once; run the SPMD program on 8 cores with device-resident inputs."""

    def __init__(self, nc):
        import jax
        import jax.numpy as jnp
        import concourse.mybir as mybir
        from concourse import bass2jax
        from concourse.bass2jax import shard_map, Mesh, PartitionSpec

        bass2jax.install_neuronx_cc_hook()
        self._jax = jax
        self._jnp = jnp
        self.nc = nc
        partition_name = nc.partition_id_tensor.name if nc.partition_id_tensor else None

        in_names, out_names, out_avals = [], [], []
        for alloc in nc.m.functions[0].allocations:
            if not isinstance(alloc, mybir.MemoryLocationSet):
                continue
            name = alloc.memorylocations[0].name
            if alloc.kind == "ExternalInput":
                if name != partition_name:
                    in_names.append(name)
            elif alloc.kind == "ExternalOutput":
                shape = tuple(alloc.tensor_shape)
                dtype = mybir.dt.np(alloc.dtype)
                out_names.append(name)
                out_avals.append(jax.core.ShapedArray(shape, dtype))
        self.param_names = list(in_names)
        self.out_names = list(out_names)
        self.out_avals = out_avals
        n_params = len(in_names)
        n_outs = len(out_names)
        bir_in_names = in_names + out_names + ([partition_name] if partition_name else [])

        devices = jax.devices()[:NCORES]
        mesh = Mesh(np.asarray(devices), ("core",))
        self.mesh = mesh
        self.pspec = PartitionSpec("core")

        def _body_k(k):
            def _body(*args):
                params = list(args[:n_params])
                outs = list(args[n_params:])
                for _ in range(k):
                    operands = params + outs
                    if partition_name is not None:
                        operands.append(bass2jax.partition_id_tensor())
                    outs = list(bass2jax._bass_exec_p.bind(
                        *operands,
                        out_avals=tuple(out_avals),
                        in_names=tuple(bir_in_names),
                        out_names=tuple(out_names),
                        lowering_input_output_aliases=(),
                        sim_require_finite=True,
                        sim_require_nnan=True,
                        nc=nc,
                    ))
                return tuple(outs)
            in_specs = (PartitionSpec("core"),) * (n_params + n_outs)
            out_specs = (PartitionSpec("core"),) * n_outs
            return jax.jit(
                shard_map(_body, mesh=mesh, in_specs=in_specs, out_specs=out_specs,
                          check_rep=False),
                keep_unused=True,
            )

        self._body_k = _body_k
        self._fns = {}
        self.dev_in = None

    def _fn(self, k):
        if k not in self._fns:
            self._fns[k] = self._body_k(k)
        return self._fns[k]

    def put_inputs(self, in_maps):
        import jax
        from jax.sharding import NamedSharding
        sh = NamedSharding(self.mesh, self.pspec)
        self.dev_in = [
            jax.device_put(
                np.concatenate([np.asarray(m[name]) for m in in_maps], axis=0), sh)
            for name in self.param_names
        ]
        self._zeros = [
            jax.device_put(np.zeros((NCORES * av.shape[0], *av.shape[1:]), av.dtype), sh)
            for av in self.out_avals
        ]

    def run(self, k=1):
        outs = self._fn(k)(*self.dev_in, *self._zeros)
        for o in outs:
            o.block_until_ready()
        return outs

    def results(self, outs):
        res = []
        for c in range(NCORES):
            d = {}
            for i, name in enumerate(self.out_names):
                av = self.out_avals[i]
                d[name] = np.asarray(outs[i]).reshape(NCORES, *av.shape)[c]
            res.append(d)
        return res

    def bench(self, iters=10, warmup=2):
        import time as _t
        for _ in range(warmup):
            self.run(1)
        walls = []
        for _ in range(iters):
            t0 = _t.time()
            self.run(1)
            walls.append((_t.time() - t0) * 1e9)
        walls.sort()
        return walls[0], walls[len(walls) // 2], walls


LAST_EXEC = None


def null_overhead_ns(iters=10):
    """Dispatch+sync overhead of a launch, measured with a trivial kernel."""
    import concourse.bacc as bacc
    import concourse.mybir as mybir
    from concourse.tile import TileContext

    key = "null"
    if key not in _PROG_CACHE:
        nc = bacc.Bacc()
        xin = nc.declare_dram_parameter("x", [128, 128], mybir.dt.float32, isOutput=False)
        out = nc.declare_dram_parameter("o", [128, 128], mybir.dt.float32, isOutput=True)
        with TileContext(nc) as tc:
            with tc.tile_pool(name="sb", bufs=1) as sb:
                t = sb.tile([128, 128], mybir.dt.float32)
                nc.sync.dma_start(out=t[:], in_=xin[:])
                nc.sync.dma_start(out=out[:], in_=t[:])
        nc.compile()
        ex = _Exec(nc)
        ex.put_inputs([{"x": np.zeros((128, 128), np.float32)} for _ in range(NCORES)])
        _PROG_CACHE[key] = ex
    ex = _PROG_CACHE[key]
    mn, med, _ = ex.bench(iters=iters)
    return mn


def _get_program(c_pad, row_tiles):
    key = (c_pad, tuple(row_tiles))
    if key not in _PROG_CACHE:
        nc = _build_program(c_pad, row_tiles)
        _PROG_CACHE[key] = _Exec(nc)
    return _PROG_CACHE[key]


def kernel(hidden_states, gate_w, w_gate, w_up, w_down, sw_gate, sw_up, sw_down):
    global LAST_EXEC
    import ml_dtypes
    BF = np.dtype(ml_dtypes.bfloat16)

    orig_shape = hidden_states.shape
    x = np.ascontiguousarray(np.asarray(hidden_states, np.float32).reshape(-1, H))
    t_tokens = x.shape[0]
    assert t_tokens == T, f"kernel compiled for T={T}, got {t_tokens}"

    # ---- host gate: softmax + top-6 (matches jax.lax.top_k tie order) ----
    logits = x @ np.asarray(gate_w, np.float32).T
    m = logits.max(axis=-1, keepdims=True)
    ex = np.exp(logits - m)
    scores = ex / ex.sum(axis=-1, keepdims=True)
    order = np.argsort(-scores, axis=-1, kind="stable")[:, :K]
    topk_w = np.take_along_axis(scores, order, axis=1)
    flat_i = order.reshape(-1).astype(np.int64)
    flat_w = topk_w.reshape(-1).astype(np.float32)
    token_ids = np.arange(T * K, dtype=np.int64) // K

    # ---- rank within expert (stable by slot index) ----
    sort_order = np.argsort(flat_i, kind="stable")
    sorted_e = flat_i[sort_order]
    counts = np.bincount(flat_i, minlength=E)
    starts = np.concatenate([[0], np.cumsum(counts)[:-1]])
    pos_sorted = np.arange(T * K) - starts[sorted_e]
    pos = np.empty(T * K, np.int64)
    pos[sort_order] = pos_sorted

    max_load = int(min(counts.max(), CAP))
    c_pad = max(128, ((max_load + 127) // 128) * 128)
    c_pad = min(c_pad, CAP)
    row_tiles = []
    r0 = 0
    while r0 < c_pad:
        row_tiles.append((r0, min(128, c_pad - r0)))
        r0 += 128

    # ---- pack per-core inputs ----

    w_gate = np.asarray(w_gate, np.float32)
    w_up = np.asarray(w_up, np.float32)
    w_down = np.asarray(w_down, np.float32)
    sw_gate = np.asarray(sw_gate, np.float32)
    sw_up = np.asarray(sw_up, np.float32)
    sw_down = np.asarray(sw_down, np.float32)

    tok_of = np.zeros((E, c_pad), np.int64)
    nrow = np.zeros(E, np.int64)
    slot_w = np.zeros((E, c_pad), np.float32)
    for e in range(E):
        cnt = int(min(counts[e], c_pad))
        seg = sort_order[starts[e]: starts[e] + cnt]
        tok_of[e, :cnt] = token_ids[seg]
        nrow[e] = cnt
        slot_w[e, :cnt] = flat_w[seg]

    # shared expert weights: replicated, packed once (chunk-major like wgu)
    swg_r = sw_gate.reshape(16, 128, IS)
    swu_r = sw_up.reshape(16, 128, IS)
    sgu_shared = {}
    for ic, (i0, ci) in enumerate(SIC):
        sgu = np.empty((16, 128, 2 * ci), BF)
        sgu[..., :ci] = swg_r[..., i0:i0 + ci]
        sgu[..., ci:] = swu_r[..., i0:i0 + ci]
        sgu_shared[f"sgu{ic}"] = sgu
    swd_bf = sw_down.astype(BF)

    in_maps = []
    for c in range(NCORES):
        eids = np.arange(c * EPC, (c + 1) * EPC)
        xt_c = np.zeros((EPC, 128, 16 * c_pad), BF)
        for jj, e in enumerate(eids):
            cnt = int(nrow[e])
            buf = np.zeros((c_pad, H), np.float32)
            buf[:cnt] = x[tok_of[e, :cnt]]
            # -> [H, c_pad] -> [16, 128, c_pad] -> [128, 16, c_pad]
            xt_c[jj] = buf.T.reshape(16, 128, c_pad).transpose(1, 0, 2).reshape(
                128, 16 * c_pad).astype(BF)
        wg_e = w_gate[eids].reshape(EPC, 16, 128, I)
        wu_e = w_up[eids].reshape(EPC, 16, 128, I)
        # this core's shared-expert tokens, token-major packed like xt
        xshc = x[c * TPC:(c + 1) * TPC].T.reshape(16, 128, TPC).transpose(
            1, 0, 2).reshape(128, 16 * TPC).astype(BF)
        im = {
            "xt": xt_c,
            "wd": w_down[eids].astype(BF),
            "scl": np.ascontiguousarray(slot_w[eids]),
            "xshc": xshc,
            "swd": swd_bf,
        }
        im.update(sgu_shared)
        for ic, (i0, ci) in enumerate(IC):
            wgu = np.empty((EPC, 16, 128, 2 * ci), BF)
            wgu[..., :ci] = wg_e[..., i0:i0 + ci]
            wgu[..., ci:] = wu_e[..., i0:i0 + ci]
            im[f"wgu{ic}"] = wgu
        in_maps.append(im)

    ex_prog = _get_program(c_pad, row_tiles)
    ex_prog.put_inputs(in_maps)
    outs = ex_prog.run(1)
    results = ex_prog.results(outs)
    LAST_EXEC = ex_prog

    # ---- host combine ----
    y_all = np.concatenate(
        [results[c]["y"].astype(np.float32) for c in range(NCORES)], axis=0)  # [E, c_pad, H]
    shared = np.concatenate(
        [results[c]["ysh"].astype(np.float64) for c in range(NCORES)], axis=0)  # [T, H]

    mask = (pos < CAP) & (pos < c_pad)
    idx = np.where(mask, pos, 0)
    rows = y_all[flat_i, idx] * mask[:, None].astype(np.float32)   # [T*K, H], pre-scaled
    moe_out = rows.reshape(T, K, H).sum(axis=1)

    out = (moe_out.astype(np.float64) + shared).astype(np.float32)
    return out.reshape(orig_shape)


# revision 16
# speedup vs baseline: 1.1522x; 1.1334x over previous
"""DeepseekMoE Trainium2 kernel (expert-parallel over 8 NeuronCores).

Strategy:
  - Host: gate (softmax + top-6), capacity dispatch bookkeeping, packing.
  - Device (SPMD, 8 cores): core c owns experts [8c, 8c+8). Grouped SwiGLU
    expert GEMMs in token-stationary orientation. All matmul operands are
    bf16 (f32 PSUM accumulation), halving HBM/transfer bytes vs f32.
    Gate+up weights for each (I-chunk, h-tile) are packed side by side on
    the host so each weight DMA is one contiguous [128, ci*2] transfer.
    Shared-expert SwiGLU is tensor-parallel over its intermediate dim
    (352 columns per core).
  - Host: weighted combine of expert rows + sum of shared partials.
"""
import os
import sys
import numpy as np

for _p in ("/opt/trn_rl_repo", "/root/.axon_site/_ro/trn_rl_repo"):
    if os.path.isdir(_p) and _p not in sys.path:
        sys.path.insert(0, _p)

E, K, H, I = 64, 6, 2048, 1408
NS = 2
IS = NS * I          # 2816
CAP = 256
T = 1024             # tokens
NCORES = 8
EPC = E // NCORES    # experts per core = 8
ISC = IS // NCORES   # shared intermediate per core = 352

IC = [(0, 512), (512, 512), (1024, 384)]          # I chunks (gate/up psum)
KD = [(0, 128), (128, 128), (256, 96)]            # shared down K tiles over 352

_PROG_CACHE = {}


def _build_program(c_pad, row_tiles, reps=1):
    """Build the SPMD Bass program. Structure depends only on (c_pad, row_tiles)."""
    import concourse.bacc as bacc
    import concourse.mybir as mybir
    from concourse.tile import TileContext
    from concourse import masks

    f32 = mybir.dt.float32
    bf16 = mybir.dt.bfloat16

    nc = bacc.Bacc()
    xt_d = nc.declare_dram_parameter("xt", [EPC, 128, 16 * c_pad], bf16, isOutput=False)
    # gate|up weights packed per (I-chunk, h-tile): one contiguous DMA each
    wgu_ds = []
    for ic, (i0, ci) in enumerate(IC):
        wgu_ds.append(nc.declare_dram_parameter(
            f"wgu{ic}", [EPC, 16, 128, 2 * ci], bf16, isOutput=False))
    wd_d = nc.declare_dram_parameter("wd", [EPC, I, H], bf16, isOutput=False)
    scl_d = nc.declare_dram_parameter("scl", [EPC, c_pad], f32, isOutput=False)
    xsh_d = nc.declare_dram_parameter("xsh", [16, 128, T], bf16, isOutput=False)
    swgu_d = nc.declare_dram_parameter("swgu", [H, 2 * ISC], bf16, isOutput=False)
    swd_d = nc.declare_dram_parameter("swd", [ISC, H], bf16, isOutput=False)
    y_d = nc.declare_dram_parameter("y", [EPC, c_pad, H], bf16, isOutput=True)
    ysh_d = nc.declare_dram_parameter("ysh", [T, H], bf16, isOutput=True)

    with TileContext(nc) as tc:
        with (
            tc.tile_pool(name="const", bufs=1) as constp,
            tc.tile_pool(name="xt", bufs=2) as xtp,
            tc.tile_pool(name="w", bufs=4) as wp,
            tc.tile_pool(name="wd", bufs=2) as wdp,
            tc.tile_pool(name="hid", bufs=3) as hidp,
            tc.tile_pool(name="hidT", bufs=2) as hidTp,
            tc.tile_pool(name="ysb", bufs=3) as ysbp,
            tc.tile_pool(name="scl", bufs=2) as sclp,
            tc.tile_pool(name="swres", bufs=1) as swresp,
            tc.tile_pool(name="psg", bufs=1, space="PSUM") as psg,
            tc.tile_pool(name="psu", bufs=1, space="PSUM") as psu,
            tc.tile_pool(name="pstr", bufs=2, space="PSUM") as pstr,
            tc.tile_pool(name="psy", bufs=1, space="PSUM") as psy,
        ):
            ident = constp.tile([128, 128], f32)
            masks.make_identity(nc, ident[:])

            for _rep in range(reps):
                _expert_shared_body(nc, tc, mybir, c_pad, row_tiles,
                                    xt_d, wgu_ds, wd_d, scl_d, xsh_d,
                                    swgu_d, swd_d, y_d, ysh_d, ident,
                                    xtp, wp, wdp, hidp, hidTp, ysbp, sclp, swresp,
                                    psg, psu, pstr, psy)
    nc.compile()
    return nc


def _expert_shared_body(nc, tc, mybir, c_pad, row_tiles,
                        xt_d, wgu_ds, wd_d, scl_d, xsh_d,
                        swgu_d, swd_d, y_d, ysh_d, ident,
                        xtp, wp, wdp, hidp, hidTp, ysbp, sclp, swresp,
                        psg, psu, pstr, psy):
    f32 = mybir.dt.float32
    bf16 = mybir.dt.bfloat16
    Silu = mybir.ActivationFunctionType.Silu

    # ---------------- expert phase ----------------
    for j in range(EPC):
        xt_sb = xtp.tile([128, 16 * c_pad], bf16, tag="xt")
        nc.sync.dma_start(out=xt_sb[:], in_=xt_d[j])
        scl_sb = sclp.tile([128, 1], f32, tag="scl")
        nc.sync.dma_start(out=scl_sb[:c_pad, 0:1], in_=scl_d[j].rearrange("(c o) -> c o", o=1))

        for (r0, rm) in row_tiles:
            hidT_sb = hidTp.tile([128, 11 * 128], bf16, tag="hidT")
            for ic, (i0, ci) in enumerate(IC):
                g_ps = psg.tile([128, 512], f32, tag="g")
                u_ps = psu.tile([128, 512], f32, tag="u")
                for h in range(16):
                    wgu_sb = wp.tile([128, 1024], bf16, tag="wgu")
                    nc.sync.dma_start(out=wgu_sb[:, :2 * ci], in_=wgu_ds[ic][j, h])
                    lhsT = xt_sb[:, h * c_pad + r0: h * c_pad + r0 + rm]
                    nc.tensor.matmul(g_ps[:rm, :ci], lhsT, wgu_sb[:, :ci],
                                     start=(h == 0), stop=(h == 15))
                    nc.tensor.matmul(u_ps[:rm, :ci], lhsT, wgu_sb[:, ci:2 * ci],
                                     start=(h == 0), stop=(h == 15))
                hid_sb = hidp.tile([128, 512], f32, tag="hid")
                nc.scalar.activation(hid_sb[:rm, :ci], g_ps[:rm, :ci], Silu)
                nc.vector.tensor_mul(hid_sb[:rm, :ci], hid_sb[:rm, :ci], u_ps[:rm, :ci])
                for t in range(ci // 128):
                    tr_ps = pstr.tile([128, 128], f32, tag="tr")
                    nc.tensor.transpose(tr_ps[:, :rm], hid_sb[:rm, t * 128:(t + 1) * 128],
                                        ident[:rm, :rm])
                    k2 = (i0 // 128) + t
                    nc.vector.tensor_copy(hidT_sb[:, k2 * 128: k2 * 128 + rm], tr_ps[:, :rm])
            # down projection
            y_ps = psy.tile([128, H], f32, tag="y")
            for k2 in range(11):
                wd_sb = wdp.tile([128, H], bf16, tag="wd")
                nc.sync.dma_start(out=wd_sb[:], in_=wd_d[j, k2 * 128:(k2 + 1) * 128, :])
                for n in range(4):
                    nc.tensor.matmul(y_ps[:rm, n * 512:(n + 1) * 512],
                                     hidT_sb[:, k2 * 128: k2 * 128 + rm],
                                     wd_sb[:, n * 512:(n + 1) * 512],
                                     start=(k2 == 0), stop=(k2 == 10))
            y_sb = ysbp.tile([128, H], bf16, tag="ysb")
            for n in range(4):
                nc.vector.tensor_scalar_mul(y_sb[:rm, n * 512:(n + 1) * 512],
                                            y_ps[:rm, n * 512:(n + 1) * 512],
                                            scl_sb[r0:r0 + rm, 0:1])
            nc.sync.dma_start(out=y_d[j, r0:r0 + rm, :], in_=y_sb[:rm, :])

    # ---------------- shared expert phase ----------------
    # resident sliced weights: swgu rows are h*128+p, cols = gate|up over ISC
    swgu_sb = swresp.tile([128, 16 * 2 * ISC], bf16, tag="swgu")
    nc.sync.dma_start(out=swgu_sb[:].rearrange("p (h i) -> p h i", h=16),
                      in_=swgu_d.rearrange("(h p) i -> p h i", p=128))
    swd_sbs = []
    for t, (k0, km) in enumerate(KD):
        sd = swresp.tile([128, H], bf16, tag=f"swd{t}", name=f"swd{t}")
        nc.sync.dma_start(out=sd[:km, :], in_=swd_d[k0:k0 + km, :])
        swd_sbs.append(sd)

    for th in range(2):  # token halves of 512
        hsh_tiles = []
        for it, (k0, km) in enumerate(KD):
            g_ps = psg.tile([128, 512], f32, tag="g")
            u_ps = psu.tile([128, 512], f32, tag="u")
            for h in range(16):
                xts = wp.tile([128, 512], bf16, tag="xts")
                nc.sync.dma_start(out=xts[:], in_=xsh_d[h, :, th * 512:(th + 1) * 512])
                lhs_g = swgu_sb[:, h * 2 * ISC + k0: h * 2 * ISC + k0 + km]
                lhs_u = swgu_sb[:, h * 2 * ISC + ISC + k0: h * 2 * ISC + ISC + k0 + km]
                nc.tensor.matmul(g_ps[:km, :], lhs_g, xts[:],
                                 start=(h == 0), stop=(h == 15))
                nc.tensor.matmul(u_ps[:km, :], lhs_u, xts[:],
                                 start=(h == 0), stop=(h == 15))
            hsh = hidp.tile([128, 512], f32, tag="hid")
            nc.scalar.activation(hsh[:km, :], g_ps[:km, :], Silu)
            nc.vector.tensor_mul(hsh[:km, :], hsh[:km, :], u_ps[:km, :])
            hshr = hidp.tile([128, 512], bf16, tag="hshr")
            nc.vector.tensor_copy(hshr[:km, :], hsh[:km, :])
            hsh_tiles.append(hshr)
        for mt in range(4):  # 128-token sub-tiles
            ysh_ps = psy.tile([128, H], f32, tag="y")
            for it, (k0, km) in enumerate(KD):
                for n in range(4):
                    nc.tensor.matmul(ysh_ps[:, n * 512:(n + 1) * 512],
                                     hsh_tiles[it][:km, mt * 128:(mt + 1) * 128],
                                     swd_sbs[it][:km, n * 512:(n + 1) * 512],
                                     start=(it == 0), stop=(it == 2))
            ysh_sb = ysbp.tile([128, H], bf16, tag="ysb")
            nc.vector.tensor_copy(ysh_sb[:], ysh_ps[:])
            nc.sync.dma_start(out=ysh_d[th * 512 + mt * 128: th * 512 + (mt + 1) * 128, :],
                              in_=ysh_sb[:])


class _Exec:
    """Compile once; run the SPMD program on 8 cores with device-resident inputs."""

    def __init__(self, nc):
        import jax
        import jax.numpy as jnp
        import concourse.mybir as mybir
        from concourse import bass2jax
        from concourse.bass2jax import shard_map, Mesh, PartitionSpec

        bass2jax.install_neuronx_cc_hook()
        self._jax = jax
        self._jnp = jnp
        self.nc = nc
        partition_name = nc.partition_id_tensor.name if nc.partition_id_tensor else None

        in_names, out_names, out_avals = [], [], []
        for alloc in nc.m.functions[0].allocations:
            if not isinstance(alloc, mybir.MemoryLocationSet):
                continue
            name = alloc.memorylocations[0].name
            if alloc.kind == "ExternalInput":
                if name != partition_name:
                    in_names.append(name)
            elif alloc.kind == "ExternalOutput":
                shape = tuple(alloc.tensor_shape)
                dtype = mybir.dt.np(alloc.dtype)
                out_names.append(name)
                out_avals.append(jax.core.ShapedArray(shape, dtype))
        self.param_names = list(in_names)
        self.out_names = list(out_names)
        self.out_avals = out_avals
        n_params = len(in_names)
        n_outs = len(out_names)
        bir_in_names = in_names + out_names + ([partition_name] if partition_name else [])

        devices = jax.devices()[:NCORES]
        mesh = Mesh(np.asarray(devices), ("core",))
        self.mesh = mesh
        self.pspec = PartitionSpec("core")

        def _body_k(k):
            def _body(*args):
                params = list(args[:n_params])
                outs = list(args[n_params:])
                for _ in range(k):
                    operands = params + outs
                    if partition_name is not None:
                        operands.append(bass2jax.partition_id_tensor())
                    outs = list(bass2jax._bass_exec_p.bind(
                        *operands,
                        out_avals=tuple(out_avals),
                        in_names=tuple(bir_in_names),
                        out_names=tuple(out_names),
                        lowering_input_output_aliases=(),
                        sim_require_finite=True,
                        sim_require_nnan=True,
                        nc=nc,
                    ))
                return tuple(outs)
            in_specs = (PartitionSpec("core"),) * (n_params + n_outs)
            out_specs = (PartitionSpec("core"),) * n_outs
            return jax.jit(
                shard_map(_body, mesh=mesh, in_specs=in_specs, out_specs=out_specs,
                          check_rep=False),
                keep_unused=True,
            )

        self._body_k = _body_k
        self._fns = {}
        self.dev_in = None

    def _fn(self, k):
        if k not in self._fns:
            self._fns[k] = self._body_k(k)
        return self._fns[k]

    def put_inputs(self, in_maps):
        import jax
        from jax.sharding import NamedSharding
        sh = NamedSharding(self.mesh, self.pspec)
        self.dev_in = [
            jax.device_put(
                np.concatenate([np.asarray(m[name]) for m in in_maps], axis=0), sh)
            for name in self.param_names
        ]
        self._zeros = [
            jax.device_put(np.zeros((NCORES * av.shape[0], *av.shape[1:]), av.dtype), sh)
            for av in self.out_avals
        ]

    def run(self, k=1):
        outs = self._fn(k)(*self.dev_in, *self._zeros)
        for o in outs:
            o.block_until_ready()
        return outs

    def results(self, outs):
        res = []
        for c in range(NCORES):
            d = {}
            for i, name in enumerate(self.out_names):
                av = self.out_avals[i]
                d[name] = np.asarray(outs[i]).reshape(NCORES, *av.shape)[c]
            res.append(d)
        return res

    def bench(self, iters=10, warmup=2):
        import time as _t
        for _ in range(warmup):
            self.run(1)
        walls = []
        for _ in range(iters):
            t0 = _t.time()
            self.run(1)
            walls.append((_t.time() - t0) * 1e9)
        walls.sort()
        return walls[0], walls[len(walls) // 2], walls


LAST_EXEC = None


def null_overhead_ns(iters=10):
    """Dispatch+sync overhead of a launch, measured with a trivial kernel."""
    import concourse.bacc as bacc
    import concourse.mybir as mybir
    from concourse.tile import TileContext

    key = "null"
    if key not in _PROG_CACHE:
        nc = bacc.Bacc()
        xin = nc.declare_dram_parameter("x", [128, 128], mybir.dt.float32, isOutput=False)
        out = nc.declare_dram_parameter("o", [128, 128], mybir.dt.float32, isOutput=True)
        with TileContext(nc) as tc:
            with tc.tile_pool(name="sb", bufs=1) as sb:
                t = sb.tile([128, 128], mybir.dt.float32)
                nc.sync.dma_start(out=t[:], in_=xin[:])
                nc.sync.dma_start(out=out[:], in_=t[:])
        nc.compile()
        ex = _Exec(nc)
        ex.put_inputs([{"x": np.zeros((128, 128), np.float32)} for _ in range(NCORES)])
        _PROG_CACHE[key] = ex
    ex = _PROG_CACHE[key]
    mn, med, _ = ex.bench(iters=iters)
    return mn


def _get_program(c_pad, row_tiles):
    key = (c_pad, tuple(row_tiles))
    if key not in _PROG_CACHE:
        nc = _build_program(c_pad, row_tiles)
        _PROG_CACHE[key] = _Exec(nc)
    return _PROG_CACHE[key]


def kernel(hidden_states, gate_w, w_gate, w_up, w_down, sw_gate, sw_up, sw_down):
    global LAST_EXEC
    import ml_dtypes
    BF = np.dtype(ml_dtypes.bfloat16)

    orig_shape = hidden_states.shape
    x = np.ascontiguousarray(np.asarray(hidden_states, np.float32).reshape(-1, H))
    t_tokens = x.shape[0]
    assert t_tokens == T, f"kernel compiled for T={T}, got {t_tokens}"

    # ---- host gate: softmax + top-6 (matches jax.lax.top_k tie order) ----
    logits = x @ np.asarray(gate_w, np.float32).T
    m = logits.max(axis=-1, keepdims=True)
    ex = np.exp(logits - m)
    scores = ex / ex.sum(axis=-1, keepdims=True)
    order = np.argsort(-scores, axis=-1, kind="stable")[:, :K]
    topk_w = np.take_along_axis(scores, order, axis=1)
    flat_i = order.reshape(-1).astype(np.int64)
    flat_w = topk_w.reshape(-1).astype(np.float32)
    token_ids = np.arange(T * K, dtype=np.int64) // K

    # ---- rank within expert (stable by slot index) ----
    sort_order = np.argsort(flat_i, kind="stable")
    sorted_e = flat_i[sort_order]
    counts = np.bincount(flat_i, minlength=E)
    starts = np.concatenate([[0], np.cumsum(counts)[:-1]])
    pos_sorted = np.arange(T * K) - starts[sorted_e]
    pos = np.empty(T * K, np.int64)
    pos[sort_order] = pos_sorted

    max_load = int(min(counts.max(), CAP))
    c_pad = max(128, ((max_load + 127) // 128) * 128)
    c_pad = min(c_pad, CAP)
    row_tiles = []
    r0 = 0
    while r0 < c_pad:
        row_tiles.append((r0, min(128, c_pad - r0)))
        r0 += 128

    # ---- pack per-core inputs ----
    x_bf = x.astype(BF)
    xT_bf = np.ascontiguousarray(x_bf.T)
    xsh = xT_bf.reshape(16, 128, T)     # replicated to all cores

    w_gate = np.asarray(w_gate, np.float32)
    w_up = np.asarray(w_up, np.float32)
    w_down = np.asarray(w_down, np.float32)
    sw_gate = np.asarray(sw_gate, np.float32)
    sw_up = np.asarray(sw_up, np.float32)
    sw_down = np.asarray(sw_down, np.float32)

    tok_of = np.zeros((E, c_pad), np.int64)
    nrow = np.zeros(E, np.int64)
    slot_w = np.zeros((E, c_pad), np.float32)
    for e in range(E):
        cnt = int(min(counts[e], c_pad))
        seg = sort_order[starts[e]: starts[e] + cnt]
        tok_of[e, :cnt] = token_ids[seg]
        nrow[e] = cnt
        slot_w[e, :cnt] = flat_w[seg]

    in_maps = []
    for c in range(NCORES):
        eids = np.arange(c * EPC, (c + 1) * EPC)
        xt_c = np.zeros((EPC, 128, 16 * c_pad), BF)
        for jj, e in enumerate(eids):
            cnt = int(nrow[e])
            buf = np.zeros((c_pad, H), np.float32)
            buf[:cnt] = x[tok_of[e, :cnt]]
            # -> [H, c_pad] -> [16, 128, c_pad] -> [128, 16, c_pad]
            xt_c[jj] = buf.T.reshape(16, 128, c_pad).transpose(1, 0, 2).reshape(
                128, 16 * c_pad).astype(BF)
        wg_e = w_gate[eids].reshape(EPC, 16, 128, I)
        wu_e = w_up[eids].reshape(EPC, 16, 128, I)
        im = {
            "xt": xt_c,
            "wd": w_down[eids].astype(BF),
            "scl": np.ascontiguousarray(slot_w[eids]),
            "xsh": xsh,
            "swgu": np.concatenate(
                [sw_gate[:, c * ISC:(c + 1) * ISC],
                 sw_up[:, c * ISC:(c + 1) * ISC]], axis=1).astype(BF),
            "swd": sw_down[c * ISC:(c + 1) * ISC, :].astype(BF),
        }
        for ic, (i0, ci) in enumerate(IC):
            wgu = np.empty((EPC, 16, 128, 2 * ci), BF)
            wgu[..., :ci] = wg_e[..., i0:i0 + ci]
            wgu[..., ci:] = wu_e[..., i0:i0 + ci]
            im[f"wgu{ic}"] = wgu
        in_maps.append(im)

    ex_prog = _get_program(c_pad, row_tiles)
    ex_prog.put_inputs(in_maps)
    outs = ex_prog.run(1)
    results = ex_prog.results(outs)
    LAST_EXEC = ex_prog

    # ---- host combine ----
    y_all = np.concatenate(
        [results[c]["y"].astype(np.float32) for c in range(NCORES)], axis=0)  # [E, c_pad, H]
    shared = np.zeros((T, H), np.float64)
    for c in range(NCORES):
        shared += results[c]["ysh"].astype(np.float64)

    mask = (pos < CAP) & (pos < c_pad)
    idx = np.where(mask, pos, 0)
    rows = y_all[flat_i, idx] * mask[:, None].astype(np.float32)   # [T*K, H], pre-scaled
    moe_out = rows.reshape(T, K, H).sum(axis=1)

    out = (moe_out.astype(np.float64) + shared).astype(np.float32)
    return out.reshape(orig_shape)
